# revision 7
# baseline (speedup 1.0000x reference)
"""MinGRU forward on 8 TRN2 NeuronCores.

Math (linear-space reformulation of the reference's log-space Heinsen scan):
    hg = x @ W_hg.T                       # [B,S,2D]
    hidden, gate = split(hg)
    z = sigmoid(gate)
    c = sigmoid(-gate)                    # = 1 - z = exp(-softplus(gate))
    g = max(hidden + 0.5, sigmoid(hidden))  # == where(h>=0, h+0.5, sigmoid(h)) exactly
    u = z * g
    h[t] = c[t] * h[t-1] + u[t]           # convex combination -> bounded, stable
    out = h

The recurrence maps directly onto the DVE `tensor_tensor_scan` instruction
(state = data0*state + data1 along the free dim, fp32 internal state).

Sharding: 8 cores = 4 batches x 2 feature-halves (512 features each).
No cross-core communication: the scan is per-feature independent.

Inputs stream as fp16 (10-bit mantissa ~ fp32r's 11): halves HBM traffic vs
fp32r so the DMA ramp never stalls the PE, at the same full-rate PE speed.
Host pre-packs x and W into the exact SBUF image ([128 partitions, ...])
so DMA lines are long and contiguous. psum/elementwise stay fp32.
"""

import numpy as np

B, S, D = 4, 4096, 1024
DH = D // 2          # features per core
N_CORES = 8
KC = 128             # contraction chunk (partition dim)
NKC = D // KC        # 8 k chunks
FC = 128             # feature chunk (psum partitions)
NFC = DH // FC       # 4 feature chunks

_CACHE = {}

# Chunk widths: 512 (one PSUM bank) in steady state, narrow at the end to
# shorten the serial post-matmul tail (act -> scan -> out-DMA).
WIDTHS = [512, 512, 512, 512, 512, 512, 512, 384, 128]
assert sum(WIDTHS) == S

CONFIG = {
    "psbufs": 4,
    "ebufs": 3,
    "split_last_scan": False,  # last chunk is narrow; split not needed
}


def _build():
    import concourse.bacc as bacc
    import concourse.tile as tile
    import concourse.mybir as mybir

    f32 = mybir.dt.float32
    f16 = mybir.dt.float16
    AF = mybir.ActivationFunctionType
    OP = mybir.AluOpType

    nc = bacc.Bacc("TRN2")
    # Pre-packed SBUF images (host does the shuffles):
    #   xP[p, k, s]     = x[b][s, k*128+p]
    #   wP[fc, p, k, e] = W row (c*DH + fc*128 + e%128) [hidden e<128 / gate]
    xP = nc.dram_tensor("xP", [KC, NKC, S], f16, kind="ExternalInput")
    wP = nc.dram_tensor("wP", [NFC, KC, NKC, 2 * FC], f16, kind="ExternalInput")
    outT = nc.dram_tensor("outT", [DH, S], f32, kind="ExternalOutput")

    with tile.TileContext(nc) as tc:
        with (
            tc.tile_pool(name="w", bufs=1) as wpool,
            tc.tile_pool(name="x", bufs=2) as xpool,
            tc.tile_pool(name="ew", bufs=CONFIG["ebufs"]) as epool,
            tc.tile_pool(name="h", bufs=2) as hpool,
            tc.tile_pool(name="ps", bufs=CONFIG["psbufs"], space="PSUM") as pspool,
        ):
            wts = []
            for fc in range(NFC):
                wtf = wpool.tile([KC, NKC, 2 * FC], f16, tag=f"w{fc}")
                wts.append(wtf)

            # Ramp: every piece lands just-in-time for the PE's consumption
            # order. W rides the ACT ring split per-(fc,k) (64KB pieces);
            # x chunk 0 is split per-k across the SP and DVE rings; outputs
            # ride the (otherwise idle) GpSimd ring so they never block the
            # in-order SP queue that carries later x chunks.
            for fc in range(NFC):
                for k in range(NKC):
                    nc.scalar.dma_start(wts[fc][:, k, :], wP[fc, :, k, :])
            xt0 = xpool.tile([KC, NKC, WIDTHS[0]], f16, tag="xt")
            for k in range(NKC):
                eng = nc.sync if k % 2 == 0 else nc.gpsimd
                eng.dma_start(xt0[:, k, :], xP[:, k, 0:WIDTHS[0]])

            hprev = [None] * NFC
            off = 0
            for sc, width in enumerate(WIDTHS):
                if sc == 0:
                    xt = xt0
                else:
                    xt = xpool.tile([KC, NKC, width], f16, tag="xt")
                    nc.sync.dma_start(xt[:], xP[:, :, off:off + width])
                for fc in range(NFC):
                    ph = pspool.tile([FC, width], f32, tag="ph")
                    pg = pspool.tile([FC, width], f32, tag="pg")
                    if sc == 0:
                        # interleave h/g per k: each 64KB weight piece feeds
                        # two back-to-back matmuls, halving the required W
                        # ring bandwidth during the ramp
                        for k in range(NKC):
                            nc.tensor.matmul(
                                ph[:], wts[fc][:, k, 0:FC], xt[:, k, :],
                                start=(k == 0), stop=(k == NKC - 1),
                            )
                            nc.tensor.matmul(
                                pg[:], wts[fc][:, k, FC:2 * FC], xt[:, k, :],
                                start=(k == 0), stop=(k == NKC - 1),
                            )
                    else:
                        for k in range(NKC):
                            nc.tensor.matmul(
                                ph[:], wts[fc][:, k, 0:FC], xt[:, k, :],
                                start=(k == 0), stop=(k == NKC - 1),
                            )
                        for k in range(NKC):
                            nc.tensor.matmul(
                                pg[:], wts[fc][:, k, FC:2 * FC], xt[:, k, :],
                                start=(k == 0), stop=(k == NKC - 1),
                            )
                    zt = epool.tile([FC, width], f32, tag="z")
                    ct = epool.tile([FC, width], f32, tag="c")
                    st = epool.tile([FC, width], f32, tag="s")
                    gt = epool.tile([FC, width], f32, tag="g")
                    ut = epool.tile([FC, width], f32, tag="u")
                    # s first: it heads the DVE critical chain (s->g->u->scan)
                    nc.scalar.activation(st[:], ph[:], AF.Sigmoid)
                    nc.scalar.activation(zt[:], pg[:], AF.Sigmoid)
                    nc.scalar.activation(ct[:], pg[:], AF.Sigmoid, scale=-1.0)
                    # g = (hidden + 0.5) max sigmoid(hidden)
                    nc.vector.scalar_tensor_tensor(
                        gt[:], ph[:], 0.5, st[:], op0=OP.add, op1=OP.max
                    )
                    nc.gpsimd.tensor_mul(ut[:], zt[:], gt[:])
                    ht = hpool.tile([FC, width], f32, tag=f"h{fc}")
                    pw = WIDTHS[sc - 1]
                    init = 0.0 if sc == 0 else hprev[fc][:, pw - 1:pw]
                    if CONFIG["split_last_scan"] and sc == len(WIDTHS) - 1:
                        hw_ = width // 2
                        nc.vector.tensor_tensor_scan(
                            ht[:, 0:hw_], ct[:, 0:hw_], ut[:, 0:hw_], init,
                            op0=OP.mult, op1=OP.add,
                        )
                        nc.gpsimd.dma_start(
                            outT[fc * FC:(fc + 1) * FC, off:off + hw_], ht[:, 0:hw_]
                        )
                        nc.vector.tensor_tensor_scan(
                            ht[:, hw_:width], ct[:, hw_:width], ut[:, hw_:width],
                            ht[:, hw_ - 1:hw_], op0=OP.mult, op1=OP.add,
                        )
                        nc.gpsimd.dma_start(
                            outT[fc * FC:(fc + 1) * FC, off + hw_:off + width],
                            ht[:, hw_:width],
                        )
                        hprev[fc] = ht
                    else:
                        nc.vector.tensor_tensor_scan(
                            ht[:], ct[:], ut[:], init, op0=OP.mult, op1=OP.add
                        )
                        hprev[fc] = ht
                        nc.gpsimd.dma_start(
                            outT[fc * FC:(fc + 1) * FC, off:off + width], ht[:]
                        )
                off += width

    nc.compile()
    return nc


def _prep_in_maps(x: np.ndarray, W_hg: np.ndarray):
    x = np.asarray(x, dtype=np.float32)
    W_hg = np.asarray(W_hg, dtype=np.float32)
    # xP[p, k, s] = x[b][s, k*128+p]
    xPs = []
    for b in range(B):
        xt = x[b].T.astype(np.float16)                      # [D, S]
        xPs.append(np.ascontiguousarray(
            xt.reshape(NKC, KC, S).transpose(1, 0, 2)))     # [KC, NKC, S]
    wPs = []
    for c in range(2):
        wp = np.empty((NFC, KC, NKC, 2 * FC), dtype=np.float16)
        for fc in range(NFC):
            rows_h = W_hg[c * DH + fc * FC:c * DH + (fc + 1) * FC]      # [FC, D]
            rows_g = W_hg[D + c * DH + fc * FC:D + c * DH + (fc + 1) * FC]
            wfc = np.empty((D, 2 * FC), dtype=np.float16)
            wfc[:, 0:FC] = rows_h.T
            wfc[:, FC:2 * FC] = rows_g.T
            wp[fc] = wfc.reshape(NKC, KC, 2 * FC).transpose(1, 0, 2)
        wPs.append(wp)
    return [{"xP": xPs[core // 2], "wP": wPs[core % 2]} for core in range(N_CORES)]


def _get_runner():
    """Build the Bass module once and cache a compiled jax callable for it.

    Mirrors bass2jax.run_bass_via_pjrt's multi-core path, but keeps the
    jitted/sharded executable so repeat kernel() calls skip re-tracing.
    """
    if "runner" in _CACHE:
        return _CACHE["runner"]

    import jax
    from jax.experimental.shard_map import shard_map
    from jax.sharding import Mesh, PartitionSpec
    from concourse import bass2jax

    if "nc" not in _CACHE:
        _CACHE["nc"] = _build()
    nc = _CACHE["nc"]
    bass2jax.install_neuronx_cc_hook()

    in_names = ["xP", "wP"]
    out_name = "outT"
    out_shape, out_dtype = (DH, S), np.float32
    partition_name = nc.partition_id_tensor.name if nc.partition_id_tensor else None

    def _body(xP, wP, zout):
        operands = [xP, wP, zout]
        if partition_name is not None:
            operands.append(bass2jax.partition_id_tensor())
        outs = bass2jax._bass_exec_p.bind(
            *operands,
            out_avals=(jax.core.ShapedArray(out_shape, out_dtype),),
            in_names=tuple(in_names + [out_name] + ([partition_name] if partition_name else [])),
            out_names=(out_name,),
            lowering_input_output_aliases=(),
            sim_require_finite=True,
            sim_require_nnan=True,
            nc=nc,
        )
        return tuple(outs)

    devices = jax.devices()[:N_CORES]
    mesh = Mesh(np.asarray(devices), ("core",))
    sharded = jax.jit(
        shard_map(
            _body, mesh=mesh,
            in_specs=(PartitionSpec("core"),) * 3,
            out_specs=(PartitionSpec("core"),),
            check_rep=False,
        ),
        donate_argnums=(2,),
        keep_unused=True,
    )

    def run(in_maps):
        concat_x = np.concatenate([m["xP"] for m in in_maps], axis=0)
        concat_w = np.concatenate([m["wP"] for m in in_maps], axis=0)
        zeros = np.zeros((N_CORES * DH, S), np.float32)
        (out_arr,) = sharded(concat_x, concat_w, zeros)
        return np.asarray(out_arr).reshape(N_CORES, DH, S)

    _CACHE["runner"] = run
    return run


def kernel(x: np.ndarray, W_hg: np.ndarray) -> np.ndarray:
    run = _get_runner()
    in_maps = _prep_in_maps(x, W_hg)
    outs = run(in_maps)

    out = np.empty((B, S, D), dtype=np.float32)
    for core in range(N_CORES):
        b, c = core // 2, core % 2
        out[b, :, c * DH:(c + 1) * DH] = outs[core].T
    return out


# revision 9
# speedup vs baseline: 1.1458x; 1.1458x over previous
"""MinGRU forward on 8 TRN2 NeuronCores.

Math (linear-space reformulation of the reference's log-space Heinsen scan):
    hg = x @ W_hg.T                       # [B,S,2D]
    hidden, gate = split(hg)
    z = sigmoid(gate)
    c = sigmoid(-gate)                    # = 1 - z = exp(-softplus(gate))
    g = max(hidden + 0.5, sigmoid(hidden))  # == where(h>=0, h+0.5, sigmoid(h)) exactly
    u = z * g
    h[t] = c[t] * h[t-1] + u[t]           # convex combination -> bounded, stable
    out = h

The recurrence maps directly onto the DVE `tensor_tensor_scan` instruction
(state = data0*state + data1 along the free dim, fp32 internal state).

Sharding: 8 cores = 4 batches x 2 feature-halves (512 features each).
No cross-core communication: the scan is per-feature independent.

Inputs stream as fp16 (10-bit mantissa ~ fp32r's 11): halves HBM traffic vs
fp32r so the DMA ramp never stalls the PE, at the same full-rate PE speed.
Host pre-packs x and W into the exact SBUF image ([128 partitions, ...])
so DMA lines are long and contiguous. psum/elementwise stay fp32.
"""

import numpy as np

B, S, D = 4, 4096, 1024
DH = D // 2          # features per core
N_CORES = 8
KC = 128             # contraction chunk (partition dim)
NKC = D // KC        # 8 k chunks
FC = 128             # feature chunk (psum partitions)
NFC = DH // FC       # 4 feature chunks

_CACHE = {}

# Chunk widths: 512 (one PSUM bank) in steady state, narrow at the end to
# shorten the serial post-matmul tail (act -> scan -> out-DMA).
WIDTHS = [512, 512, 512, 512, 512, 512, 512, 384, 128]
assert sum(WIDTHS) == S

CONFIG = {
    "psbufs": 4,
    "ebufs": 3,
    "split_last_scan": False,  # last chunk is narrow; split not needed
}


def _build():
    import concourse.bacc as bacc
    import concourse.tile as tile
    import concourse.mybir as mybir

    f32 = mybir.dt.float32
    f16 = mybir.dt.float16
    AF = mybir.ActivationFunctionType
    OP = mybir.AluOpType

    nc = bacc.Bacc("TRN2")
    # Pre-packed SBUF images (host does the shuffles):
    #   xP[p, k, s]     = x[b][s, k*128+p]
    #   wP[fc, p, k, e] = W row (c*DH + fc*128 + e%128) [hidden e<128 / gate]
    xP = nc.dram_tensor("xP", [KC, NKC, S], f16, kind="ExternalInput")
    wP = nc.dram_tensor("wP", [NFC, KC, NKC, 2 * FC], f16, kind="ExternalInput")
    outT = nc.dram_tensor("outT", [DH, S], f32, kind="ExternalOutput")

    with tile.TileContext(nc) as tc:
        with (
            tc.tile_pool(name="w", bufs=1) as wpool,
            tc.tile_pool(name="x", bufs=2) as xpool,
            tc.tile_pool(name="ew", bufs=CONFIG["ebufs"]) as epool,
            tc.tile_pool(name="h", bufs=2) as hpool,
            tc.tile_pool(name="ps", bufs=CONFIG["psbufs"], space="PSUM") as pspool,
        ):
            wts = []
            for fc in range(NFC):
                wtf = wpool.tile([KC, NKC, 2 * FC], f16, tag=f"w{fc}")
                wts.append(wtf)

            # Ramp layout. Each dma_start costs ~0.6us on the issuing
            # engine's instruction stream, so: W rides the SP ring (idle
            # stream) in fine-to-coarse pieces matching the PE's consumption
            # order; x chunk 0 is split per-k across the GpSimd and ACT
            # rings; later x chunks follow W on the SP ring; outputs ride
            # the GpSimd ring so they never block the SP queue.
            nc.sync.dma_start(wts[0][:, 0, :], wP[0, :, 0, :])
            nc.sync.dma_start(wts[0][:, 1, :], wP[0, :, 1, :])
            nc.sync.dma_start(wts[0][:, 2:4, :], wP[0, :, 2:4, :])
            nc.sync.dma_start(wts[0][:, 4:8, :], wP[0, :, 4:8, :])
            for fc in range(1, NFC):
                nc.sync.dma_start(wts[fc][:], wP[fc])
            xt0 = xpool.tile([KC, NKC, WIDTHS[0]], f16, tag="xt")
            for k in range(NKC):
                eng = nc.gpsimd if k % 2 == 0 else nc.scalar
                eng.dma_start(xt0[:, k, :], xP[:, k, 0:WIDTHS[0]])

            hprev = [None] * NFC
            off = 0
            for sc, width in enumerate(WIDTHS):
                if sc == 0:
                    xt = xt0
                else:
                    xt = xpool.tile([KC, NKC, width], f16, tag="xt")
                    nc.sync.dma_start(xt[:], xP[:, :, off:off + width])
                for fc in range(NFC):
                    ph = pspool.tile([FC, width], f32, tag="ph")
                    pg = pspool.tile([FC, width], f32, tag="pg")
                    if sc == 0:
                        # interleave h/g per k: each 64KB weight piece feeds
                        # two back-to-back matmuls, halving the required W
                        # ring bandwidth during the ramp
                        for k in range(NKC):
                            nc.tensor.matmul(
                                ph[:], wts[fc][:, k, 0:FC], xt[:, k, :],
                                start=(k == 0), stop=(k == NKC - 1),
                            )
                            nc.tensor.matmul(
                                pg[:], wts[fc][:, k, FC:2 * FC], xt[:, k, :],
                                start=(k == 0), stop=(k == NKC - 1),
                            )
                    else:
                        for k in range(NKC):
                            nc.tensor.matmul(
                                ph[:], wts[fc][:, k, 0:FC], xt[:, k, :],
                                start=(k == 0), stop=(k == NKC - 1),
                            )
                        for k in range(NKC):
                            nc.tensor.matmul(
                                pg[:], wts[fc][:, k, FC:2 * FC], xt[:, k, :],
                                start=(k == 0), stop=(k == NKC - 1),
                            )
                    zt = epool.tile([FC, width], f32, tag="z")
                    ct = epool.tile([FC, width], f32, tag="c")
                    st = epool.tile([FC, width], f32, tag="s")
                    gt = epool.tile([FC, width], f32, tag="g")
                    ut = epool.tile([FC, width], f32, tag="u")
                    # s first: it heads the DVE critical chain (s->g->u->scan)
                    nc.scalar.activation(st[:], ph[:], AF.Sigmoid)
                    nc.scalar.activation(zt[:], pg[:], AF.Sigmoid)
                    nc.scalar.activation(ct[:], pg[:], AF.Sigmoid, scale=-1.0)
                    # g = (hidden + 0.5) max sigmoid(hidden)
                    nc.vector.scalar_tensor_tensor(
                        gt[:], ph[:], 0.5, st[:], op0=OP.add, op1=OP.max
                    )
                    nc.vector.tensor_mul(ut[:], zt[:], gt[:])
                    ht = hpool.tile([FC, width], f32, tag=f"h{fc}")
                    pw = WIDTHS[sc - 1]
                    init = 0.0 if sc == 0 else hprev[fc][:, pw - 1:pw]
                    if CONFIG["split_last_scan"] and sc == len(WIDTHS) - 1:
                        hw_ = width // 2
                        nc.vector.tensor_tensor_scan(
                            ht[:, 0:hw_], ct[:, 0:hw_], ut[:, 0:hw_], init,
                            op0=OP.mult, op1=OP.add,
                        )
                        nc.gpsimd.dma_start(
                            outT[fc * FC:(fc + 1) * FC, off:off + hw_], ht[:, 0:hw_]
                        )
                        nc.vector.tensor_tensor_scan(
                            ht[:, hw_:width], ct[:, hw_:width], ut[:, hw_:width],
                            ht[:, hw_ - 1:hw_], op0=OP.mult, op1=OP.add,
                        )
                        nc.gpsimd.dma_start(
                            outT[fc * FC:(fc + 1) * FC, off + hw_:off + width],
                            ht[:, hw_:width],
                        )
                        hprev[fc] = ht
                    else:
                        nc.vector.tensor_tensor_scan(
                            ht[:], ct[:], ut[:], init, op0=OP.mult, op1=OP.add
                        )
                        hprev[fc] = ht
                        nc.gpsimd.dma_start(
                            outT[fc * FC:(fc + 1) * FC, off:off + width], ht[:]
                        )
                off += width

    nc.compile()
    return nc


def _prep_in_maps(x: np.ndarray, W_hg: np.ndarray):
    x = np.asarray(x, dtype=np.float32)
    W_hg = np.asarray(W_hg, dtype=np.float32)
    # xP[p, k, s] = x[b][s, k*128+p]
    xPs = []
    for b in range(B):
        xt = x[b].T.astype(np.float16)                      # [D, S]
        xPs.append(np.ascontiguousarray(
            xt.reshape(NKC, KC, S).transpose(1, 0, 2)))     # [KC, NKC, S]
    wPs = []
    for c in range(2):
        wp = np.empty((NFC, KC, NKC, 2 * FC), dtype=np.float16)
        for fc in range(NFC):
            rows_h = W_hg[c * DH + fc * FC:c * DH + (fc + 1) * FC]      # [FC, D]
            rows_g = W_hg[D + c * DH + fc * FC:D + c * DH + (fc + 1) * FC]
            wfc = np.empty((D, 2 * FC), dtype=np.float16)
            wfc[:, 0:FC] = rows_h.T
            wfc[:, FC:2 * FC] = rows_g.T
            wp[fc] = wfc.reshape(NKC, KC, 2 * FC).transpose(1, 0, 2)
        wPs.append(wp)
    return [{"xP": xPs[core // 2], "wP": wPs[core % 2]} for core in range(N_CORES)]


def _get_runner():
    """Build the Bass module once and cache a compiled jax callable for it.

    Mirrors bass2jax.run_bass_via_pjrt's multi-core path, but keeps the
    jitted/sharded executable so repeat kernel() calls skip re-tracing.
    """
    if "runner" in _CACHE:
        return _CACHE["runner"]

    import jax
    from jax.experimental.shard_map import shard_map
    from jax.sharding import Mesh, PartitionSpec
    from concourse import bass2jax

    if "nc" not in _CACHE:
        _CACHE["nc"] = _build()
    nc = _CACHE["nc"]
    bass2jax.install_neuronx_cc_hook()

    in_names = ["xP", "wP"]
    out_name = "outT"
    out_shape, out_dtype = (DH, S), np.float32
    partition_name = nc.partition_id_tensor.name if nc.partition_id_tensor else None

    def _body(xP, wP, zout):
        operands = [xP, wP, zout]
        if partition_name is not None:
            operands.append(bass2jax.partition_id_tensor())
        outs = bass2jax._bass_exec_p.bind(
            *operands,
            out_avals=(jax.core.ShapedArray(out_shape, out_dtype),),
            in_names=tuple(in_names + [out_name] + ([partition_name] if partition_name else [])),
            out_names=(out_name,),
            lowering_input_output_aliases=(),
            sim_require_finite=True,
            sim_require_nnan=True,
            nc=nc,
        )
        return tuple(outs)

    devices = jax.devices()[:N_CORES]
    mesh = Mesh(np.asarray(devices), ("core",))
    sharded = jax.jit(
        shard_map(
            _body, mesh=mesh,
            in_specs=(PartitionSpec("core"),) * 3,
            out_specs=(PartitionSpec("core"),),
            check_rep=False,
        ),
        donate_argnums=(2,),
        keep_unused=True,
    )

    def run(in_maps):
        concat_x = np.concatenate([m["xP"] for m in in_maps], axis=0)
        concat_w = np.concatenate([m["wP"] for m in in_maps], axis=0)
        zeros = np.zeros((N_CORES * DH, S), np.float32)
        (out_arr,) = sharded(concat_x, concat_w, zeros)
        return np.asarray(out_arr).reshape(N_CORES, DH, S)

    _CACHE["runner"] = run
    return run


def kernel(x: np.ndarray, W_hg: np.ndarray) -> np.ndarray:
    run = _get_runner()
    in_maps = _prep_in_maps(x, W_hg)
    outs = run(in_maps)

    out = np.empty((B, S, D), dtype=np.float32)
    for core in range(N_CORES):
        b, c = core // 2, core % 2
        out[b, :, c * DH:(c + 1) * DH] = outs[core].T
    return out
